# revision 9
# baseline (speedup 1.0000x reference)
"""Multi-head self-attention (B=2, S=2048, D=512, H=8) on 8 TRN2 NeuronCores.

Sharding: tensor-parallel over the 8 heads - core h computes head h for both
batch elements; the host sums the 8 row-parallel output-projection partials
and adds bias.

Per-core dataflow (bf16 projections/scores, fp8 DoubleRow for attn @ V):
  xT bf16 [512, 4096] (d-major)
  Q,K = W @ xT (bf16) -> [128, 4096] bf16 (rows 0-63 batch0, 64-127 batch1)
  V   = x @ Wv (bf16) -> fp8 hi (x8) + fp8 residual, j-major, +ones col (=8)
  per super (b, s = 512 query positions), per j-group (256 key positions):
    ST[j, i] = K^T Q     (2 bf16 matmuls into a [128, 2, 512] PSUM pair)
    P = exp(ST) -> fp8   (ScalarE exact for most groups; DVE computes a few
                          via an exponent-bit trick: u8 = ST*8*log2e + 56.5,
                          bitcast to e4m3 ~= exp with ~4% rel err)
    oT[dd, i] += V8^T P + Vr^T P  (two fp8 DoubleRow matmuls, 0.5 cyc/row,
                          contracting 256 keys each; row 64 = denominators)
  drain: oT -> bf16 SBUF; den row -> columns via K=1 matmuls; reciprocal;
  po = oT^T @ WoT (bf16); scale rows by 1/den (GPSIMD) -> bf16 out DMA.
Host: out = sum_h partial_h + bo + Wo @ bv.
"""

import sys

for _p in ("/opt/trn_rl_repo", "/root/.axon_site/_ro/trn_rl_repo"):
    if _p not in sys.path:
        sys.path.insert(0, _p)

import numpy as np
import ml_dtypes

import concourse.bass as bass
import concourse.mybir as mybir
import concourse.tile as tile
from concourse import bacc
from concourse.bass_utils import run_bass_kernel_spmd

F32 = mybir.dt.float32
BF16 = mybir.dt.bfloat16
FP8 = mybir.dt.float8e4
U8 = mybir.dt.uint8
EXP = mybir.ActivationFunctionType.Exp
DR = mybir.MatmulPerfMode.DoubleRow
MULT = mybir.AluOpType.mult
ADD = mybir.AluOpType.add
SUB = mybir.AluOpType.subtract

B, S, D, H, DEPTH = 2, 2048, 512, 8, 64
N = B * S  # 4096 total positions
KC = D // 128  # 4 contraction chunks
NJC = S // 128  # 16 j-chunks per batch
NJG = NJC // 2  # 8 j-groups (pairs of chunks) per batch
AV = 8.0  # fp8 V scale; ones column = AV so num/den ratio is unchanged

# exp(x) ~= bitcast_e4m3(u8(x * 8*log2e + 56.5)); byte 56 == 1.0
SCH_MULT = float(8.0 * np.log2(np.e))
SCH_BIAS = 56.0
# j-groups per super computed on DVE via the bit trick (rest: ScalarE exact)
DVE_EXP_GROUPS = (3, 6)


def build_nc():
    nc = bacc.Bacc("TRN2", target_bir_lowering=False)
    xT = nc.dram_tensor("xT", [D, N], BF16, kind="ExternalInput").ap()
    wq = nc.dram_tensor("wq", [D, 2 * DEPTH], BF16, kind="ExternalInput").ap()
    wk = nc.dram_tensor("wk", [D, 2 * DEPTH], BF16, kind="ExternalInput").ap()
    wv = nc.dram_tensor("wv", [D, DEPTH], BF16, kind="ExternalInput").ap()
    wo = nc.dram_tensor("wo", [DEPTH, D], BF16, kind="ExternalInput").ap()
    bq = nc.dram_tensor("bq", [128, 1], F32, kind="ExternalInput").ap()
    bk = nc.dram_tensor("bk", [128, 1], F32, kind="ExternalInput").ap()
    out = nc.dram_tensor("out", [B, S, D], BF16, kind="ExternalOutput").ap()
    out_r = out.rearrange("b (s ic p) m -> b s p ic m", ic=4, p=128)

    with tile.TileContext(nc) as tc:
        with (
            tc.tile_pool(name="sb_const", bufs=1) as sb_const,
            tc.tile_pool(name="sb_x", bufs=1) as sb_x,
            tc.tile_pool(name="sb_qk", bufs=1) as sb_qk,
            tc.tile_pool(name="sb_v", bufs=1) as sb_v,
            tc.tile_pool(name="sb_p", bufs=4) as sb_p,
            tc.tile_pool(name="sb_ot", bufs=2) as sb_ot,
            tc.tile_pool(name="sb_rr", bufs=2) as sb_rr,
            tc.tile_pool(name="sb_ob", bufs=2) as sb_ob,
        ):
            # ---- DMA loads, first-needed-first
            xT_r = xT.rearrange("(c p) n -> p c n", p=128)
            xts = [None] * (N // 512)

            def load_xt(t):
                xt_t = sb_x.tile([128, KC, 512], BF16, tag=f"xt{t}", name=f"xt{t}")
                nc.sync.dma_start(out=xt_t[:], in_=xT_r[:, :, bass.ds(t * 512, 512)])
                xts[t] = xt_t

            load_xt(0)
            wq_sb = sb_const.tile([128, KC, 2 * DEPTH], BF16, tag="wq")
            wk_sb = sb_const.tile([128, KC, 2 * DEPTH], BF16, tag="wk")
            wv_sb = sb_const.tile([128, KC, DEPTH], BF16, tag="wv")
            nc.sync.dma_start(out=wq_sb[:], in_=wq.rearrange("(c p) m -> p c m", p=128))
            nc.sync.dma_start(out=wk_sb[:], in_=wk.rearrange("(c p) m -> p c m", p=128))
            bq_sb = sb_const.tile([128, 1], F32, tag="bq")
            nc.sync.dma_start(out=bq_sb[:], in_=bq)
            bk_sb = sb_const.tile([128, 1], F32, tag="bk")
            nc.sync.dma_start(out=bk_sb[:], in_=bk)
            nc.sync.dma_start(out=wv_sb[:], in_=wv.rearrange("(c p) m -> p c m", p=128))
            for t in (1, 2, 3, 4, 5, 6, 7):
                load_xt(t)
            wo_sb = sb_const.tile([DEPTH, D], BF16, tag="wo")
            nc.sync.dma_start(out=wo_sb[:], in_=wo)
            ones_sb = sb_const.tile([128, 1], BF16, tag="ones")
            nc.vector.memset(ones_sb[:], 1.0)

            # Warm the ScalarE exp table while the first DMAs run.
            warm = sb_const.tile([1, 1], F32, tag="warm")
            nc.vector.memset(warm, 0.0)
            nc.scalar.activation(out=warm, in_=warm, func=EXP)

            def xt_slice(pos0, width):
                t, off = divmod(pos0, 512)
                assert off + width <= 512
                return xts[t][:, :, bass.ds(off, width)]

            q_sb = sb_qk.tile([128, S], BF16, tag="q")
            k_sb = sb_qk.tile([128, S], BF16, tag="k")
            # V hi/residual: [128 j, b, jg, pair, 80] fp8 (col 64 = ones*AV / 0;
            # pair stride 80: DoubleRow LdWeights needs middle-dim step % 16 == 0)
            v8_t = sb_v.tile([128, B, NJG, 2, 80], FP8, tag="v8")
            vr_t = sb_v.tile([128, B, NJG, 2, 80], FP8, tag="vr")
            nc.vector.memset(v8_t[:, :, :, :, 64], AV)
            nc.vector.memset(vr_t[:, :, :, :, 64], 0.0)

            def emit_qk_proj(pool, dst, w_sb, b_sb, b, nchunk):
                """One [64, 512] projection chunk of Q or K (batch b)."""
                rows = bass.ds(b * 64, 64)
                pt = pool.tile(
                    [128, 512], F32, tag="small", bufs=2,
                    name=f"pt_{dst.name}_{b}_{nchunk}",
                )
                for c in range(KC):
                    nc.tensor.matmul(
                        out=pt[:],
                        lhsT=w_sb[:, c, :],
                        rhs=xt_slice(b * S + nchunk * 512, 512)[:, c, :],
                        start=(c == 0),
                        stop=(c == KC - 1),
                    )
                nc.vector.tensor_scalar_add(
                    out=dst[rows, bass.ds(nchunk * 512, 512)],
                    in0=pt[rows, :],
                    scalar1=b_sb[rows, :],
                )

            def emit_v_proj(pool, b, jc):
                vt = pool.tile([128, 64], F32, tag="small", bufs=2, name=f"vt_{b}_{jc}")
                for c in range(KC):
                    nc.tensor.matmul(
                        out=vt[:],
                        lhsT=xt_slice(b * S + jc * 128, 128)[:, c, :],
                        rhs=wv_sb[:, c, :],
                        start=(c == 0),
                        stop=(c == KC - 1),
                    )
                jg, half = divmod(jc, 2)
                v8_d = v8_t[:, b, jg, half, 0:64]
                # v8 cast on ScalarE (Copy with scale), residual on DVE
                nc.scalar.activation(
                    out=v8_d, in_=vt[:],
                    func=mybir.ActivationFunctionType.Copy, scale=AV,
                )
                nc.vector.scalar_tensor_tensor(
                    out=vr_t[:, b, jg, half, 0:64],
                    in0=vt[:],
                    scalar=AV,
                    in1=v8_d,
                    op0=MULT,
                    op1=SUB,
                )

            def drain_stages(pool, sup, ot_tile):
                """Super drain split into 6 short stages (one per j-group slot)."""
                b, s = sup
                ot_sb = sb_ot.tile([DEPTH + 1, 512], BF16, tag="ot", name=f"otsb_{b}_{s}")
                rr = sb_rr.tile([128, 4], F32, tag="rr", name=f"rr_{b}_{s}")
                ob = sb_ob.tile([128, 4, 512], BF16, tag="ob", name=f"ob_{b}_{s}")

                def stage0():
                    nc.vector.tensor_copy(out=ot_sb[:], in_=ot_tile[:])

                def stage1():
                    rs = pool.tile([128, 4], F32, tag="small", bufs=2, name=f"rs_{b}_{s}")
                    for c in range(4):
                        # K=1 matmul: transpose denominator row chunk to a column
                        nc.tensor.matmul(
                            out=rs[:, c : c + 1],
                            lhsT=ot_sb[64:65, bass.ds(c * 128, 128)],
                            rhs=ones_sb[64:65, :],
                            start=True,
                            stop=True,
                        )
                    nc.vector.reciprocal(out=rr[:], in_=rs[:])

                def mk_po(ic):
                    def stage():
                        po = pool.tile(
                            [128, 512], F32, tag="small", bufs=2, name=f"po_{b}_{s}_{ic}"
                        )
                        nc.tensor.matmul(
                            out=po[:],
                            lhsT=ot_sb[0:DEPTH, bass.ds(ic * 128, 128)],
                            rhs=wo_sb[:],
                            start=True,
                            stop=True,
                        )
                        if ic % 2 == 0:
                            # GPSIMD can't touch PSUM; split the psum->sbuf
                            # scale between ScalarE and DVE.
                            nc.scalar.activation(
                                out=ob[:, ic, :],
                                in_=po[:],
                                func=mybir.ActivationFunctionType.Copy,
                                scale=rr[:, ic : ic + 1],
                            )
                        else:
                            nc.vector.tensor_scalar_mul(
                                out=ob[:, ic, :], in0=po[:], scalar1=rr[:, ic : ic + 1]
                            )
                        if ic == 3:
                            nc.sync.dma_start(out=out_r[b, s], in_=ob[:])

                    return stage

                return [stage0, stage1, mk_po(0), mk_po(1), mk_po(2), mk_po(3)]

            # ---- main loop: 8 supers, 8 j-groups each; fillers interleaved.
            with tc.tile_pool(name="psum", bufs=1, space="PSUM") as pool:
                # filler work emitted before (pre) / after (post) the exp of
                # group g of super u
                fillers = {}
                post_fillers = {}

                def add_filler(u, g, fn, post=False):
                    d = post_fillers if post else fillers
                    d.setdefault((u, g), []).append(fn)

                def qk(dst, w, bias, b, n):
                    return lambda: emit_qk_proj(pool, dst, w, bias, b, n)

                def vp(b, jc):
                    return lambda: emit_v_proj(pool, b, jc)

                # u0: V(b0,0..3) right after the first two exps (oT g0/g1 deps);
                # K(b0,n) before group 2n; V(b0,4..15) spread over g2..7.
                add_filler(0, 0, vp(0, 0), post=True)
                add_filler(0, 0, vp(0, 1), post=True)
                add_filler(0, 1, vp(0, 2), post=True)
                add_filler(0, 1, vp(0, 3), post=True)
                add_filler(0, 2, qk(k_sb, wk_sb, bk_sb, 0, 1))
                add_filler(0, 4, qk(k_sb, wk_sb, bk_sb, 0, 2))
                add_filler(0, 6, qk(k_sb, wk_sb, bk_sb, 0, 3))
                for jc in range(4, 16):
                    add_filler(0, 2 + (jc - 4) // 2, vp(0, jc))
                add_filler(1, 4, qk(q_sb, wq_sb, bq_sb, 0, 2))
                add_filler(2, 4, qk(q_sb, wq_sb, bq_sb, 0, 3))
                add_filler(2, 5, qk(k_sb, wk_sb, bk_sb, 1, 0))
                add_filler(2, 6, qk(k_sb, wk_sb, bk_sb, 1, 1))
                add_filler(2, 7, qk(k_sb, wk_sb, bk_sb, 1, 2))
                add_filler(3, 3, qk(k_sb, wk_sb, bk_sb, 1, 3))
                add_filler(3, 4, qk(q_sb, wq_sb, bq_sb, 1, 0))
                for jc in range(0, 12):  # V(b1, 0..11) during u3 g5..7
                    add_filler(3, 5 + jc // 4, vp(1, jc))
                for jc in range(12, 16):
                    add_filler(4, 3 + (jc - 12) // 2, vp(1, jc))
                add_filler(4, 6, qk(q_sb, wq_sb, bq_sb, 1, 1))
                add_filler(5, 4, qk(q_sb, wq_sb, bq_sb, 1, 2))
                add_filler(6, 4, qk(q_sb, wq_sb, bq_sb, 1, 3))

                pending = []  # drain stages of the previous super
                for u in range(8):
                    b, s = u // 4, u % 4
                    if u == 0:
                        emit_qk_proj(pool, q_sb, wq_sb, bq_sb, 0, 0)
                        emit_qk_proj(pool, k_sb, wk_sb, bk_sb, 0, 0)
                    elif u == 1:
                        emit_qk_proj(pool, q_sb, wq_sb, bq_sb, 0, 1)

                    ot_t = pool.tile(
                        [DEPTH + 1, 512], F32, tag="ot", bufs=2, name=f"ot_{b}_{s}"
                    )
                    for g in range(NJG):
                        for fn in fillers.pop((u, g), ()):
                            fn()
                        if pending:  # one drain stage of the previous super
                            pending.pop(0)()
                        st2 = pool.tile(
                            [128, 2, 512], F32, tag="st", bufs=2, name=f"st_{u}_{g}"
                        )
                        isl = bass.ds(s * 512, 512)
                        rows = bass.ds(b * 64, 64)
                        for t in range(2):
                            jsl = bass.ds((2 * g + t) * 128, 128)
                            nc.tensor.matmul(
                                out=st2[:, t, :],
                                lhsT=k_sb[rows, jsl],
                                rhs=q_sb[rows, isl],
                                start=True,
                                stop=True,
                            )
                        p2 = sb_p.tile([128, 2, 512], FP8, tag="p", name=f"p_{u}_{g}")
                        if g in DVE_EXP_GROUPS:
                            nc.vector.tensor_scalar(
                                out=p2[:].bitcast(U8),
                                in0=st2[:],
                                scalar1=SCH_MULT,
                                scalar2=SCH_BIAS,
                                op0=MULT,
                                op1=ADD,
                            )
                        else:
                            nc.scalar.activation(out=p2[:], in_=st2[:], func=EXP)
                        for fn in post_fillers.pop((u, g), ()):
                            fn()
                        nc.tensor.matmul(
                            out=ot_t[:],
                            lhsT=v8_t[:, b, g, :, 0:65],
                            rhs=p2[:],
                            start=(g == 0),
                            stop=False,
                            perf_mode=DR,
                            skip_group_check=True,
                        )
                        nc.tensor.matmul(
                            out=ot_t[:],
                            lhsT=vr_t[:, b, g, :, 0:65],
                            rhs=p2[:],
                            start=False,
                            stop=(g == NJG - 1),
                            perf_mode=DR,
                            skip_group_check=True,
                        )
                    assert not any(k[0] == u for k in fillers), f"unused fillers {u}"
                    assert not pending, f"unconsumed drain stages at super {u}"
                    pending = drain_stages(pool, (b, s), ot_t)
                for fn in pending:
                    fn()
    nc.compile()
    return nc


_NC_CACHE = None


def _get_nc():
    global _NC_CACHE
    if _NC_CACHE is None:
        _NC_CACHE = build_nc()
    return _NC_CACHE


def kernel(x, Wq, bq, Wk, bk, Wv, bv, Wo, bo):
    x = np.asarray(x, dtype=np.float32)
    Wq, bq, Wk, bk, Wv, bv, Wo, bo = (
        np.asarray(a, dtype=np.float32) for a in (Wq, bq, Wk, bk, Wv, bv, Wo, bo)
    )
    bf = ml_dtypes.bfloat16

    xT = np.ascontiguousarray(x.reshape(N, D).T.astype(bf))  # [512, 4096]
    scale = 1.0 / np.sqrt(np.float32(DEPTH))

    in_maps = []
    for h in range(H):
        sl = slice(h * DEPTH, (h + 1) * DEPTH)
        in_maps.append(
            {
                "xT": xT,
                "wq": np.ascontiguousarray(np.tile((Wq[sl, :] * scale).T, (1, 2)).astype(bf)),
                "wk": np.ascontiguousarray(np.tile(Wk[sl, :].T, (1, 2)).astype(bf)),
                "wv": np.ascontiguousarray(Wv[sl, :].T.astype(bf)),
                "wo": np.ascontiguousarray(Wo[:, sl].T.astype(bf)),
                "bq": np.tile(bq[sl] * scale, 2).reshape(128, 1).copy(),
                "bk": np.tile(bk[sl], 2).reshape(128, 1).copy(),
            }
        )

    nc = _get_nc()
    res = run_bass_kernel_spmd(nc, in_maps, core_ids=list(range(H)))

    acc = res.results[0]["out"].astype(np.float32).copy()
    for h in range(1, H):
        acc += res.results[h]["out"].astype(np.float32)
    acc += bo + Wo @ bv
    return acc
